# revision 1
# baseline (speedup 1.0000x reference)
"""Deformable convolution (mmcv v1, deformable_groups=1) on 8 Trainium2 cores.

Problem: x [4, 64, 64, 64], offset [4, 18, 64, 64], weight [64, 64, 3, 3]
         -> out [4, 64, 64, 64]  (3x3, stride 1, pad 1, dil 1, fp32)

Sharding: 8 cores = 4 samples x 2 spatial halves (32 output rows each);
the small weight is replicated. Each core runs the same program (SPMD) on
per-core input slices and returns its [64, 2048] output slab.

Per-core program:
  1. PE-transpose x [64c, 4096hw] and write a "vertical pair" image to DRAM:
     x2[j] = [xtrow(j-64), xtrow(j)] (128 ch-rows per entry, zero guard bands)
     so all 4 bilinear corners of any sample point sit in one contiguous
     256-element span.
  2. PE-transpose the offset slice to pixel-on-partition layout; DVE computes
     bilinear corner weights (out-of-bounds corners get weight 0) and an int32
     span-start table clip(y0*64 + x0 + 65, 0, 4161)*128.
  3. Per (tap, pixel-tile): one gpsimd indirect DMA (per-partition dynamic
     offsets) gathers the 1KB corner span for 128 pixels. DVE multiplies by
     corner weights via a c-broadcast AP and folds the 4 corners -> cols.
  4. PE-transpose cols to [ck, pix]; GEMM against W2[ck, o] (ck = k*64+c,
     zero-padded to 640) with PSUM accumulation -> out [64o, 2048pix].
"""
import numpy as np
import concourse.bass as bass
import concourse.bacc as bacc
import concourse.mybir as mybir
from concourse.ap import AP
from concourse.bass import IndirectOffsetOnAxis
from concourse import tile
from concourse.bass_utils import run_bass_kernel_spmd

F32 = mybir.dt.float32
I32 = mybir.dt.int32
AOP = mybir.AluOpType

B = 4
C = 64
O = 64
K = 9
H = W = 64
NPIX = 2048
XT_ROWS = 4164  # x2 buffer rows: 64 front guards + shifted pair rows


def dram_view(t, offset, dims):
    """Raw AP on a DRAM tensor handle (flat element offsets)."""
    return AP(t, offset, [list(d) for d in dims])


def tview(tile, free_off, free_dims, nparts=128, part0=0):
    """AP view of an SBUF pool tile: dim0 = [pitch, nparts], then free dims."""
    base = tile[:]
    pitch = base.ap[0][0]
    return AP(
        base.tensor,
        base.offset + part0 * pitch + free_off,
        [[pitch, nparts]] + [list(d) for d in free_dims],
    )


def _build(nc, tc, outs, ins):
    x = ins["x"]            # [64, 4096]
    off = ins["off"]        # [18, 2048]
    w2 = ins["w2"]          # [128, 5, 64]
    baseq = ins["baseq"]    # [128, 2, 144]  (0: y, 1: x+1)  free = pt*9+k
    ident = ins["ident"]    # [128, 128]
    out = outs["out"]       # [64, 2048]

    x2_dram = nc.dram_tensor("x2_scratch", [XT_ROWS, 2 * C], F32, kind="Internal")

    with (
        tc.tile_pool(name="const", bufs=1) as constp,
        tc.tile_pool(name="stage", bufs=1) as stagep,
        tc.tile_pool(name="work", bufs=2) as workp,
        tc.tile_pool(name="gath", bufs=1) as gathp,
        tc.tile_pool(name="pp", bufs=2) as ppp,
        tc.tile_pool(name="ps", bufs=2, space="PSUM") as psp,
        tc.tile_pool(name="psg", bufs=2, space="PSUM") as psgp,
    ):
        # ---- load constants / inputs ----
        ident_sb = constp.tile([128, 128], F32, tag="ident")
        nc.sync.dma_start(ident_sb[:], ident[:])
        x_sb = constp.tile([64, 4096], F32, tag="x")
        nc.sync.dma_start(x_sb[:], x[:])
        w2_sb = constp.tile([128, 5, O], F32, tag="w2")
        nc.sync.dma_start(w2_sb[:], w2[:])
        off_sb = constp.tile([18, NPIX], F32, tag="off")
        nc.sync.dma_start(off_sb[:], off[:])
        baseq_sb = constp.tile([128, 2, 144], F32, tag="baseq")
        nc.sync.dma_start(baseq_sb[:], baseq[:])

        # ---- S2: x -> x2 DRAM: x2[j] = [xtrow(j-64), xtrow(j)] ----
        # xtrow(r): zero guard at r=0, image row r-1 for r in 1..4096, zero 4097+.
        # Span at j = S+64 covers corners (tb0lr0, tb1lr0, tb0lr1, tb1lr1).
        zbig = constp.tile([128, C], F32, tag="zbig")
        nc.vector.memset(zbig[:], 0.0)
        # zero fills: first half rows 0..64 and 4161..4163; second half 0, 4097..4163
        nc.sync.dma_start(x2_dram[0:65, 0:C], zbig[0:65, :])
        nc.sync.dma_start(x2_dram[4161:4164, 0:C], zbig[0:3, :])
        nc.sync.dma_start(x2_dram[0:1, C : 2 * C], zbig[0:1, :])
        nc.sync.dma_start(x2_dram[4097:4164, C : 2 * C], zbig[0:67, :])
        for i in range(32):
            pst = psp.tile([128, 64], F32, tag="ps")
            nc.tensor.transpose(
                pst[:], x_sb[:, i * 128 : (i + 1) * 128], ident_sb[:64, :64]
            )
            xts = workp.tile([128, 64], F32, tag="xts")
            nc.scalar.copy(xts[:], pst[:])
            # first half: x2[65 + i*128 + p][0:64] = xtrow(1 + i*128 + p)
            nc.sync.dma_start(x2_dram[65 + i * 128 : 65 + (i + 1) * 128, 0:C], xts[:])
            # second half: x2[1 + i*128 + p][64:128] = xtrow(1 + i*128 + p)
            nc.sync.dma_start(x2_dram[1 + i * 128 : 1 + (i + 1) * 128, C : 2 * C], xts[:])

        # ---- S3: offset transpose (q-layout only) ----
        offq_sb = stagep.tile([128, 16, 18], F32, tag="offq")
        for pt in range(16):
            pso = psp.tile([128, 18], F32, tag="ps")
            nc.tensor.transpose(
                pso[:], off_sb[:, pt * 128 : (pt + 1) * 128], ident_sb[:18, :18]
            )
            nc.scalar.copy(offq_sb[:, pt, :], pso[:])

        # ---- helpers for the [128, 144] math ----
        def floor_(dst, src, tmp_i32, tag):
            # dst = floor(src): t = int-cast(src) back-cast, dst = t - (t > src)
            nc.vector.tensor_copy(tmp_i32[:], src)       # f32 -> i32
            tf = workp.tile([128, 144], F32, tag=tag + "_tf")
            nc.vector.tensor_copy(tf[:], tmp_i32[:])     # i32 -> f32
            gt = workp.tile([128, 144], F32, tag=tag + "_gt")
            nc.vector.tensor_tensor(gt[:], tf[:], src, AOP.is_gt)
            nc.vector.tensor_tensor(dst, tf[:], gt[:], AOP.subtract)

        def offv(parity):
            return tview(offq_sb, parity, [[18, 16], [2, 9]])

        # ---- S4: weights in q-layout [128 q, (pt, k)] ----
        pyq = workp.tile([128, 144], F32, tag="pyq")
        nc.vector.tensor_tensor(pyq[:], offv(0), baseq_sb[:, 0, :], AOP.add)
        pxq = workp.tile([128, 144], F32, tag="pxq")
        nc.vector.tensor_tensor(pxq[:], offv(1), baseq_sb[:, 1, :], AOP.add)
        ti32 = workp.tile([128, 144], mybir.dt.int32, tag="ti32")
        y0q = workp.tile([128, 144], F32, tag="y0q")
        floor_(y0q[:], pyq[:], ti32, "fy")
        x1q = workp.tile([128, 144], F32, tag="x1q")
        floor_(x1q[:], pxq[:], ti32, "fx")
        lyq = workp.tile([128, 144], F32, tag="lyq")
        nc.vector.tensor_tensor(lyq[:], pyq[:], y0q[:], AOP.subtract)
        lxq = workp.tile([128, 144], F32, tag="lxq")
        nc.vector.tensor_tensor(lxq[:], pxq[:], x1q[:], AOP.subtract)

        def valid(src, lo, hi, tag):
            a = workp.tile([128, 144], F32, tag=tag + "_a")
            nc.vector.tensor_scalar(a[:], src, float(lo), None, AOP.is_ge)
            b = workp.tile([128, 144], F32, tag=tag + "_b")
            nc.vector.tensor_scalar(b[:], src, float(hi), None, AOP.is_le)
            nc.vector.tensor_tensor(a[:], a[:], b[:], AOP.mult)
            return a

        vy0 = valid(y0q[:], 0, 63, "vy0")
        vy1 = valid(y0q[:], -1, 62, "vy1")
        vx0 = valid(x1q[:], 1, 64, "vx0")
        vx1 = valid(x1q[:], 0, 63, "vx1")

        wy0 = workp.tile([128, 144], F32, tag="wy0")
        nc.vector.tensor_scalar(wy0[:], lyq[:], -1.0, 1.0, AOP.mult, AOP.add)
        nc.vector.tensor_tensor(wy0[:], wy0[:], vy0[:], AOP.mult)
        wy1 = workp.tile([128, 144], F32, tag="wy1")
        nc.vector.tensor_tensor(wy1[:], lyq[:], vy1[:], AOP.mult)
        wx0 = workp.tile([128, 144], F32, tag="wx0")
        nc.vector.tensor_scalar(wx0[:], lxq[:], -1.0, 1.0, AOP.mult, AOP.add)
        nc.vector.tensor_tensor(wx0[:], wx0[:], vx0[:], AOP.mult)
        wx1 = workp.tile([128, 144], F32, tag="wx1")
        nc.vector.tensor_tensor(wx1[:], lxq[:], vx1[:], AOP.mult)

        # wt [128, k 9, pt 16, lr 2, tb 2]
        wt_sb = stagep.tile([128, K, 16, 2, 2], F32, tag="wt")
        wys = [wy0, wy1]
        wxs = [wx0, wx1]
        for tb in range(2):
            for lr in range(2):
                # src iteration (pt, k): [128][16 (9)][9 (1)]
                # dst offset = k*64 + pt*4 + lr*2 + tb : [128][16 (4)][9 (64)]
                dst = tview(wt_sb, lr * 2 + tb, [[4, 16], [64, 9]])
                tmp = workp.tile([128, 144], F32, tag="wtmp")
                nc.vector.tensor_tensor(tmp[:], wys[tb][:], wxs[lr][:], AOP.mult)
                srcv = tview(tmp, 0, [[9, 16], [1, 9]])
                nc.vector.tensor_copy(dst, srcv)

        # ---- S5: gather table: x2 elem start = clip(y0*64+x1+64, 0, 4161)*128 ----
        Sq = workp.tile([128, 144], F32, tag="Sq")
        nc.vector.scalar_tensor_tensor(Sq[:], y0q[:], 64.0, x1q[:], AOP.mult, AOP.add)
        idx32 = stagep.tile([128, K, 16], I32, tag="idx32")
        Tq = workp.tile([128, 144], F32, tag="Tq")
        nc.vector.tensor_scalar(Tq[:], Sq[:], 64.0, None, AOP.add)
        nc.vector.tensor_scalar(Tq[:], Tq[:], 0.0, 4161.0, AOP.max, AOP.min)
        dstT = tview(idx32, 0, [[1, 16], [16, 9]])
        srcT = tview(Tq, 0, [[9, 16], [1, 9]])
        nc.vector.tensor_scalar(dstT, srcT, 128.0, None, AOP.mult)

        # ---- S7/S8: gather + combine per tap ----
        # span of 256 elems from x2 at row S covers corners in order
        # (lr0tb0, lr0tb1, lr1tb0, lr1tb1) as [lr 2][tb 2][c 64]
        cols_sb = stagep.tile([128, 16, K, C], F32, tag="cols")
        x2_flat = dram_view(x2_dram, 0, [[1, XT_ROWS * 2 * C], [1, 1]])
        for k in range(K):
            G = gathp.tile([128, 16, 4 * C], F32, tag="G")
            for pt in range(16):
                idxv = tview(idx32, k * 16 + pt, [[1, 1]])
                gi = nc.gpsimd.indirect_dma_start(
                    G[:, pt, :], None, x2_flat,
                    IndirectOffsetOnAxis(ap=idxv, axis=0),
                )
                gi.ins.single_packet = True
            P = ppp.tile([128, 4096], F32, tag="P")
            # iteration (pt 16, lr 2, tb 2, c 64); wt [k][pt][lr][tb]
            wv = tview(wt_sb, k * 64, [[4, 16], [1, 4], [0, C]])
            gv = tview(G, 0, [[256, 16], [64, 4], [1, C]])
            pv = tview(P, 0, [[256, 16], [64, 4], [1, C]])
            nc.vector.tensor_tensor(pv, gv, wv, AOP.mult)
            # fold tb in place: P[., pt, lr, 0, :] += P[., pt, lr, 1, :]
            pa = tview(P, 0, [[256, 16], [128, 2], [1, C]])
            pb = tview(P, C, [[256, 16], [128, 2], [1, C]])
            nc.vector.tensor_tensor(pa, pa, pb, AOP.add)
            # fold lr -> cols[:, :, k, :]
            qv0 = tview(P, 0, [[256, 16], [1, C]])
            qv1 = tview(P, 2 * C, [[256, 16], [1, C]])
            dstc = tview(cols_sb, k * C, [[K * C, 16], [1, C]])
            nc.vector.tensor_tensor(dstc, qv0, qv1, AOP.add)

        # ---- S9/S10: transpose cols + GEMM ----
        for ch in range(4):
            colsT = workp.tile([128, 5, 512], F32, tag="colsT")
            for ptl in range(4):
                pt = ch * 4 + ptl
                for t in range(5):
                    wdt = 128 if t < 4 else 64
                    srcc = tview(cols_sb, pt * K * C + t * 128, [[1, wdt]])
                    pstc = psgp.tile([128, 128], F32, tag="pstc")
                    nc.tensor.transpose(pstc[:wdt, :], srcc, ident_sb[:])
                    nc.scalar.copy(
                        colsT[:wdt, t, ptl * 128 : (ptl + 1) * 128], pstc[:wdt, :]
                    )
                # zero the ck-pad rows of the last tile
                nc.vector.memset(colsT[64:128, 4, ptl * 128 : (ptl + 1) * 128], 0.0)
            pso = psgp.tile([64, 512], F32, tag="pso_out")
            for t in range(5):
                nc.tensor.matmul(
                    pso[:],
                    w2_sb[:, t, :],
                    colsT[:, t, :],
                    start=(t == 0),
                    stop=(t == 4),
                )
            ost = workp.tile([64, 512], F32, tag="ost")
            nc.scalar.copy(ost[:], pso[:])
            nc.sync.dma_start(out[:, ch * 512 : (ch + 1) * 512], ost[:])

def _host_prep_w2(weight):
    w = weight.reshape(O, C, K)
    W2 = np.transpose(w, (2, 1, 0)).reshape(K * C, O)
    W2p = np.zeros((640, O), np.float32)
    W2p[: K * C] = W2
    return np.ascontiguousarray(W2p.reshape(5, 128, O).transpose(1, 0, 2))


def _base_tiles(h):
    ki = np.arange(K) // 3
    kj = np.arange(K) % 3
    q = np.arange(128)[:, None, None]
    pt = np.arange(16)[None, :, None]
    k = np.arange(K)[None, None, :]
    p = pt * 128 + q
    baseq_y = (h * 32 + p // 64 + ki[k] - 1).astype(np.float32)
    baseq_x1 = (p % 64 + kj[k]).astype(np.float32)
    return np.ascontiguousarray(
        np.stack([baseq_y.reshape(128, 144), baseq_x1.reshape(128, 144)], 1)
    )


_PROGRAM = None
_last_in_maps = None


def _get_program():
    global _PROGRAM
    if _PROGRAM is None:
        nc = bacc.Bacc(
            "TRN2",
            target_bir_lowering=False,
            debug=False,
            enable_asserts=False,
            num_devices=8,
        )
        ins = {
            "x": nc.dram_tensor("x", [C, H * W], F32, kind="ExternalInput").ap(),
            "off": nc.dram_tensor("off", [18, NPIX], F32, kind="ExternalInput").ap(),
            "w2": nc.dram_tensor("w2", [128, 5, O], F32, kind="ExternalInput").ap(),
            "baseq": nc.dram_tensor(
                "baseq", [128, 2, 144], F32, kind="ExternalInput"
            ).ap(),
            "ident": nc.dram_tensor(
                "ident", [128, 128], F32, kind="ExternalInput"
            ).ap(),
        }
        outs = {
            "out": nc.dram_tensor("out", [O, NPIX], F32, kind="ExternalOutput").ap()
        }
        with tile.TileContext(nc) as tc:
            _build(nc, tc, outs, ins)
        nc.compile()
        _PROGRAM = nc
    return _PROGRAM


def _kernel_device(x, offset, weight):
    global _last_in_maps
    x = np.ascontiguousarray(np.asarray(x, np.float32))
    offset = np.ascontiguousarray(np.asarray(offset, np.float32))
    weight = np.ascontiguousarray(np.asarray(weight, np.float32))
    nc = _get_program()
    w2 = _host_prep_w2(weight)
    ident = np.eye(128, dtype=np.float32)
    bases = [_base_tiles(0), _base_tiles(1)]
    in_maps = []
    for core in range(8):
        b, h = core // 2, core % 2
        in_maps.append(
            {
                "x": np.ascontiguousarray(x[b].reshape(C, H * W)),
                "off": np.ascontiguousarray(
                    offset[b, :, h * 32 : (h + 1) * 32, :].reshape(18, NPIX)
                ),
                "w2": w2,
                "baseq": bases[h],
                "ident": ident,
            }
        )
    _last_in_maps = in_maps
    res = run_bass_kernel_spmd(nc, in_maps, list(range(8)))
    out = np.empty((B, O, H, W), np.float32)
    for core in range(8):
        b, h = core // 2, core % 2
        out[b, :, h * 32 : (h + 1) * 32, :] = res.results[core]["out"].reshape(
            O, 32, W
        )
    return out


def _kernel_numpy(x, offset, weight):
    """Exact CPU fallback (same math as the device kernel)."""
    out = np.zeros((B, O, H, W), np.float32)
    Kh = Kw = 3
    ki = np.repeat(np.arange(Kh), Kw)
    kj = np.tile(np.arange(Kw), Kh)
    for b in range(B):
        xf = x[b].reshape(C, H * W)
        off = offset[b].reshape(K, 2, H, W)
        ho = np.arange(H)[None, :, None]
        wo = np.arange(W)[None, None, :]
        py = ho - 1 + ki[:, None, None] + off[:, 0]
        px = wo - 1 + kj[:, None, None] + off[:, 1]
        y0 = np.floor(py).astype(np.int64)
        x0 = np.floor(px).astype(np.int64)
        ly = (py - y0).astype(np.float32)
        lx = (px - x0).astype(np.float32)
        cols = np.zeros((C, K, H * W), np.float32)
        for dy in (0, 1):
            for dx in (0, 1):
                yy = y0 + dy
                xx = x0 + dx
                valid = (yy >= 0) & (yy < H) & (xx >= 0) & (xx < W)
                idx = np.clip(yy, 0, H - 1) * W + np.clip(xx, 0, W - 1)
                wgt = (ly if dy else 1 - ly) * (lx if dx else 1 - lx) * valid
                cols += xf[:, idx.reshape(K, -1)] * wgt.reshape(1, K, -1)
        out[b] = (
            weight.reshape(O, C, K).transpose(0, 2, 1).reshape(O, K * C)
            @ cols.transpose(1, 0, 2).reshape(K * C, H * W)
        ).reshape(O, H, W)
    return out


_KERNEL_FAILED = False


def kernel(x, offset, weight):
    global _KERNEL_FAILED
    x = np.ascontiguousarray(np.asarray(x, np.float32))
    offset = np.ascontiguousarray(np.asarray(offset, np.float32))
    weight = np.ascontiguousarray(np.asarray(weight, np.float32))
    if not _KERNEL_FAILED:
        try:
            return _kernel_device(x, offset, weight)
        except Exception as e:
            import sys

            print(f"device kernel failed ({type(e).__name__}: {e}); "
                  "falling back to CPU", file=sys.stderr)
            _KERNEL_FAILED = True
    return _kernel_numpy(x, offset, weight)



# revision 11
# speedup vs baseline: 1.7377x; 1.7377x over previous
"""Deformable convolution (mmcv v1, deformable_groups=1) on 8 Trainium2 cores.

Problem: x [4, 64, 64, 64], offset [4, 18, 64, 64], weight [64, 64, 3, 3]
         -> out [4, 64, 64, 64]  (3x3, stride 1, pad 1, dil 1, fp32)

Sharding: 8 cores = 4 samples x 2 spatial halves (32 output rows each);
the small weight is replicated. Each core runs the same program (SPMD) on
per-core input slices and returns its [64, 2048] output slab.

Per-core program (fp16 gather/GEMM pipeline, fp32 offset math + accum):
  1. Host preps x2 fp16 [4164, 128]: x2[r] = [xpix(r-65), xpix(r-1)] with
     zero guard bands, so the 4 bilinear corners of any sample point form
     one contiguous 256-element span at row S = y0*64 + x0 + 65.
  2. DVE computes bilinear corner weights (OOB corners weighted 0) from the
     offsets (pixel-on-partition layout, host pre-transposed) and an int16
     span-row table clip(y0*64+x1+64, 0, 4161).
  3. Per tap: ONE batched dma_gather (2048 idxs x 512B spans) pulls all
     corner spans; DVE multiplies by corner weights (c-broadcast AP) and
     folds the 4 corners -> cols [128 pix, 16, 64c] fp16.
  4. DMA-xbar transpose cols -> colsT [64c, 2048 pix]; PE accumulates
     out += W_k^T @ colsT into PSUM over the 9 taps (fp32 accum).
"""
import numpy as np
import concourse.bass as bass
import concourse.bacc as bacc
import concourse.mybir as mybir
from concourse.ap import AP
from concourse import tile
from concourse.bass_utils import run_bass_kernel_spmd
from concourse.library_config import mlp

F16 = mybir.dt.float16
F32 = mybir.dt.float32
I16 = mybir.dt.int16
AOP = mybir.AluOpType

B = 4
C = 64
O = 64
K = 9
H = W = 64
NPIX = 2048
XT_ROWS = 4164  # x2 rows: 64+1 front guards + image + back guards


def dram_view(t, offset, dims):
    """Raw AP on a DRAM tensor handle (flat element offsets)."""
    return AP(t, offset, [list(d) for d in dims])


def tview(tile, free_off, free_dims, nparts=128, part0=0):
    """AP view of an SBUF pool tile: dim0 = [pitch, nparts], then free dims."""
    base = tile[:]
    pitch = base.ap[0][0]
    return AP(
        base.tensor,
        base.offset + part0 * pitch + free_off,
        [[pitch, nparts]] + [list(d) for d in free_dims],
    )


def _build(nc, tc, outs, ins):
    x2 = ins["x2"]          # [4164, 128] f16 (DRAM; gather source)
    offq = ins["offq"]      # [128, 16, 18] f32 (pixel-on-partition offsets)
    w2 = ins["w2"]          # [64, 9, 64] f16 (w2[c,k,o] = weight[o,c,k])
    baseq = ins["baseq"]    # [128, 2, 144] f32 (0: y-base, 1: x-base+1)
    repl = ins["repl"]      # [128, 128] f32 (repl[q,i] = 1 iff q%16 == i%16)
    mmask = ins["mmask"]    # [128, 8] f32 (mmask[q,m] = 1 iff q//16 == m)
    out = outs["out"]       # [64, 2048] f32

    with (
        tc.tile_pool(name="const", bufs=1) as constp,
        tc.tile_pool(name="work", bufs=2) as workp,
        tc.tile_pool(name="gath", bufs=3) as gathp,
        tc.tile_pool(name="pp", bufs=2) as ppp,
        tc.tile_pool(name="cols", bufs=2) as colsp,
        tc.tile_pool(name="colsT", bufs=2) as colsTp,
        tc.tile_pool(name="ps", bufs=1, space="PSUM") as psp,
        tc.tile_pool(name="psq", bufs=1, space="PSUM") as psqp,
    ):
        nc.gpsimd.load_library(mlp)

        # ---- load inputs ----
        offq_sb = constp.tile([128, 16, 18], F32, tag="offq")
        nc.sync.dma_start(offq_sb[:], offq[:])
        w2_sb = constp.tile([64, K, O], F16, tag="w2")
        nc.sync.dma_start(w2_sb[:], w2[:])
        baseq_sb = constp.tile([128, 2, 144], F32, tag="baseq")
        nc.sync.dma_start(baseq_sb[:], baseq[:])
        repl_sb = constp.tile([128, 128], F32, tag="repl")
        nc.sync.dma_start(repl_sb[:], repl[:])
        mmask_sb = constp.tile([128, 8], F32, tag="mmask")
        nc.sync.dma_start(mmask_sb[:], mmask[:])

        # ---- helpers for the [128, 144] math (free = pt*9 + k) ----
        def floor_(dst, src, tmp_i32, tag):
            # dst = floor(src): t = int-cast(src) back-cast, dst = t - (t > src)
            nc.vector.tensor_copy(tmp_i32[:], src)       # f32 -> i32
            tf = workp.tile([128, 144], F32, tag=tag + "_tf")
            nc.vector.tensor_copy(tf[:], tmp_i32[:])     # i32 -> f32
            gt = workp.tile([128, 144], F32, tag=tag + "_gt")
            nc.vector.tensor_tensor(gt[:], tf[:], src, AOP.is_gt)
            nc.vector.tensor_tensor(dst, tf[:], gt[:], AOP.subtract)

        def offv(parity):
            return tview(offq_sb, parity, [[18, 16], [2, 9]])

        # ---- sampling positions ----
        pyq = workp.tile([128, 144], F32, tag="pyq")
        nc.vector.tensor_tensor(pyq[:], offv(0), baseq_sb[:, 0, :], AOP.add)
        pxq = workp.tile([128, 144], F32, tag="pxq")
        nc.vector.tensor_tensor(pxq[:], offv(1), baseq_sb[:, 1, :], AOP.add)
        ti32 = workp.tile([128, 144], mybir.dt.int32, tag="ti32")
        y0q = workp.tile([128, 144], F32, tag="y0q")
        floor_(y0q[:], pyq[:], ti32, "fy")
        x1q = workp.tile([128, 144], F32, tag="x1q")
        floor_(x1q[:], pxq[:], ti32, "fx")

        # ---- span-row table first (gathers depend only on this) ----
        # row S = clip(y0*64 + x1 + 64, 0, 4161); table layout (k, pt*8+m)
        Sq = workp.tile([128, 144], F32, tag="Sq")
        nc.vector.scalar_tensor_tensor(Sq[:], y0q[:], 64.0, x1q[:], AOP.mult, AOP.add)
        Tq = workp.tile([128, 144], F32, tag="Tq")
        nc.vector.tensor_scalar(Tq[:], Sq[:], 64.0, None, AOP.add)
        nc.vector.tensor_scalar(Tq[:], Tq[:], 0.0, 4161.0, AOP.max, AOP.min)

        # Partition shuffle via PE: scatter Tq into per-m-group slots, then
        # one replication matmul (repl[q,i] = 1 iff q%16 == i%16) makes the
        # full table replicated across all 8 partition groups.
        # table[16c + j%16, k, j//16] = S(pixel j, tap k), j//16 = pt*8 + q//16
        # TqS[q, (k, pt, m)] = Tq[q, (pt,k)] * (q//16 == m)  (one DVE op)
        TqS = constp.tile([128, K, 16, 8], F32, tag="TqS")
        tq_b = tview(Tq, 0, [[1, 9], [9, 16], [0, 8]])
        mk_b = tview(mmask_sb, 0, [[0, 9], [0, 16], [1, 8]])
        nc.vector.tensor_tensor(TqS[:], tq_b, mk_b, AOP.mult)
        idxs_sb = constp.tile([128, K, 128], I16, tag="idxs")
        psq = psqp.tile([128, K * 128], F32, tag="psq")
        for lo, hi in ((0, 512), (512, 1024), (1024, K * 128)):
            nc.tensor.matmul(
                psq[:, lo:hi],
                repl_sb[:],
                tview(TqS, lo, [[1, hi - lo]]),
                start=True,
                stop=True,
            )
            nc.vector.tensor_copy(
                tview(idxs_sb, lo, [[1, hi - lo]]), psq[:, lo:hi]
            )

        # ---- corner weights ----
        lyq = workp.tile([128, 144], F32, tag="lyq")
        nc.vector.tensor_tensor(lyq[:], pyq[:], y0q[:], AOP.subtract)
        lxq = workp.tile([128, 144], F32, tag="lxq")
        nc.vector.tensor_tensor(lxq[:], pxq[:], x1q[:], AOP.subtract)

        def valid(src, lo, hi, tag):
            a = workp.tile([128, 144], F32, tag=tag + "_a")
            nc.vector.tensor_scalar(a[:], src, float(lo), None, AOP.is_ge)
            b = workp.tile([128, 144], F32, tag=tag + "_b")
            nc.vector.tensor_scalar(b[:], src, float(hi), None, AOP.is_le)
            nc.vector.tensor_tensor(a[:], a[:], b[:], AOP.mult)
            return a

        vy0 = valid(y0q[:], 0, 63, "vy0")
        vy1 = valid(y0q[:], -1, 62, "vy1")
        vx0 = valid(x1q[:], 1, 64, "vx0")
        vx1 = valid(x1q[:], 0, 63, "vx1")

        wy0 = workp.tile([128, 144], F32, tag="wy0")
        nc.vector.tensor_scalar(wy0[:], lyq[:], -1.0, 1.0, AOP.mult, AOP.add)
        nc.vector.tensor_tensor(wy0[:], wy0[:], vy0[:], AOP.mult)
        wy1 = workp.tile([128, 144], F32, tag="wy1")
        nc.vector.tensor_tensor(wy1[:], lyq[:], vy1[:], AOP.mult)
        wx0 = workp.tile([128, 144], F32, tag="wx0")
        nc.vector.tensor_scalar(wx0[:], lxq[:], -1.0, 1.0, AOP.mult, AOP.add)
        nc.vector.tensor_tensor(wx0[:], wx0[:], vx0[:], AOP.mult)
        wx1 = workp.tile([128, 144], F32, tag="wx1")
        nc.vector.tensor_tensor(wx1[:], lxq[:], vx1[:], AOP.mult)

        # wt [128, k 9, pt 16, lr 2, tb 2] fp16
        wt_sb = constp.tile([128, K, 16, 2, 2], F16, tag="wt")
        wys = [wy0, wy1]
        wxs = [wx0, wx1]
        for tb in range(2):
            for lr in range(2):
                # src iteration (pt, k): [128][16 (9)][9 (1)]
                # dst offset = k*64 + pt*4 + lr*2 + tb : [128][16 (4)][9 (64)]
                dst = tview(wt_sb, lr * 2 + tb, [[4, 16], [64, 9]])
                tmp = workp.tile([128, 144], F32, tag="wtmp")
                nc.vector.tensor_tensor(tmp[:], wys[tb][:], wxs[lr][:], AOP.mult)
                srcv = tview(tmp, 0, [[9, 16], [1, 9]])
                nc.vector.tensor_copy(dst, srcv)

        # ---- per-tap: gather + weight/fold + transpose + GEMM accumulate ----
        pso = psp.tile([64, NPIX], F32, tag="pso_out")
        x2_ap = dram_view(x2, 0, [[2 * C, XT_ROWS - 1], [1, 4 * C]])
        out_sb = constp.tile([64, NPIX], F32, tag="out_sb")
        for k in range(K):
            G = gathp.tile([128, 16, 4 * C], F16, tag="G")
            # 2x 1024-idx gathers (2048 in one instruction overruns the
            # SWDGE descriptor ring and kills the NEFF)
            for s in range(2):
                nc.gpsimd.dma_gather(
                    G[:, s * 8 : (s + 1) * 8, :],
                    x2_ap,
                    idxs_sb[:, k, s * 64 : (s + 1) * 64],
                    NPIX // 2,
                    NPIX // 2,
                    4 * C,
                    elem_step=2 * C,
                )
            P = ppp.tile([128, 4096], F16, tag="P")
            # iteration (pt 16, lr 2, tb 2, c 64); wt [k][pt][lr][tb]
            wv = tview(wt_sb, k * 64, [[4, 16], [1, 4], [0, C]])
            gv = tview(G, 0, [[256, 16], [64, 4], [1, C]])
            pv = tview(P, 0, [[256, 16], [64, 4], [1, C]])
            nc.vector.tensor_tensor(pv, gv, wv, AOP.mult)
            # fold tb in place: P[., pt, lr, 0, :] += P[., pt, lr, 1, :]
            pa = tview(P, 0, [[256, 16], [128, 2], [1, C]])
            pb = tview(P, C, [[256, 16], [128, 2], [1, C]])
            nc.vector.tensor_tensor(pa, pa, pb, AOP.add)
            # fold lr -> cols [128 pix, 16 pt, 128 (c<64 data, rest pad)]
            cols = colsp.tile([128, 16, 128], F16, tag="cols")
            qv0 = tview(P, 0, [[256, 16], [1, C]])
            qv1 = tview(P, 2 * C, [[256, 16], [1, C]])
            cv = tview(cols, 0, [[128, 16], [1, C]])
            nc.vector.tensor_tensor(cv, qv0, qv1, AOP.add)
            # xbar transpose: colsT[p, ch, j] = cols[j, ch*128 + p]
            # -> partitions 0:64 hold channel c, middle dim = pixel tile
            colsT = colsTp.tile([128, 16, 128], F16, tag="colsT")
            nc.sync.dma_start(colsT[:], cols[:], transpose=True)
            # GEMM accumulate: out[o, pix] += sum_c w2[c,k,o] * colsT[c, pix]
            for t in range(4):
                nc.tensor.matmul(
                    pso[:, t * 512 : (t + 1) * 512],
                    w2_sb[:, k, :],
                    tview(colsT, t * 512, [[1, 512]], nparts=64),
                    start=(k == 0),
                    stop=(k == K - 1),
                )
                if k == K - 1:
                    nc.scalar.copy(
                        out_sb[:, t * 512 : (t + 1) * 512],
                        pso[:, t * 512 : (t + 1) * 512],
                    )
        nc.sync.dma_start(out[:], out_sb[:])


def _host_prep_w2(weight):
    # w2[c, k, o] = weight[o, c, k]
    w = weight.reshape(O, C, K)
    return np.ascontiguousarray(np.transpose(w, (1, 2, 0))).astype(np.float16)


def _base_tiles(h):
    ki = np.arange(K) // 3
    kj = np.arange(K) % 3
    q = np.arange(128)[:, None, None]
    pt = np.arange(16)[None, :, None]
    k = np.arange(K)[None, None, :]
    p = pt * 128 + q
    baseq_y = (h * 32 + p // 64 + ki[k] - 1).astype(np.float32)
    baseq_x1 = (p % 64 + kj[k]).astype(np.float32)
    return np.ascontiguousarray(
        np.stack([baseq_y.reshape(128, 144), baseq_x1.reshape(128, 144)], 1)
    )


_PROGRAM = None
_last_in_maps = None


def _get_program():
    global _PROGRAM
    if _PROGRAM is None:
        nc = bacc.Bacc(
            "TRN2",
            target_bir_lowering=False,
            debug=False,
            enable_asserts=False,
            num_devices=8,
        )
        ins = {
            "x2": nc.dram_tensor("x2", [XT_ROWS, 2 * C], F16, kind="ExternalInput"),
            "offq": nc.dram_tensor(
                "offq", [128, 16, 18], F32, kind="ExternalInput"
            ).ap(),
            "w2": nc.dram_tensor("w2", [C, K, O], F16, kind="ExternalInput").ap(),
            "baseq": nc.dram_tensor(
                "baseq", [128, 2, 144], F32, kind="ExternalInput"
            ).ap(),
            "repl": nc.dram_tensor(
                "repl", [128, 128], F32, kind="ExternalInput"
            ).ap(),
            "mmask": nc.dram_tensor(
                "mmask", [128, 8], F32, kind="ExternalInput"
            ).ap(),
        }
        outs = {
            "out": nc.dram_tensor("out", [O, NPIX], F32, kind="ExternalOutput").ap()
        }
        with tile.TileContext(nc) as tc:
            _build(nc, tc, outs, ins)
        nc.compile()
        _PROGRAM = nc
    return _PROGRAM


def _host_prep_x2(xb):
    # x2[r] = [xpix(r-65), xpix(r-1)], zero guards
    xp = np.ascontiguousarray(xb.reshape(C, H * W).T).astype(np.float16)
    x2 = np.zeros((XT_ROWS, 2 * C), np.float16)
    x2[65 : 65 + H * W, 0:C] = xp
    x2[1 : 1 + H * W, C : 2 * C] = xp
    return x2


def _kernel_device(x, offset, weight):
    global _last_in_maps
    nc = _get_program()
    w2 = _host_prep_w2(weight)
    bases = [_base_tiles(0), _base_tiles(1)]
    x2s = [_host_prep_x2(x[b]) for b in range(B)]
    q = np.arange(128)
    repl = (q[:, None] % 16 == q[None, :] % 16).astype(np.float32)
    mmask = (q[:, None] // 16 == np.arange(8)[None, :]).astype(np.float32)
    in_maps = []
    for core in range(8):
        b, h = core // 2, core % 2
        offs = offset[b, :, h * 32 : (h + 1) * 32, :].reshape(18, NPIX)
        offq = np.ascontiguousarray(
            offs.T.reshape(16, 128, 18).transpose(1, 0, 2)
        )
        in_maps.append(
            {
                "x2": x2s[b],
                "offq": offq,
                "w2": w2,
                "baseq": bases[h],
                "repl": repl,
                "mmask": mmask,
            }
        )
    _last_in_maps = in_maps
    res = run_bass_kernel_spmd(nc, in_maps, list(range(8)))
    out = np.empty((B, O, H, W), np.float32)
    for core in range(8):
        b, h = core // 2, core % 2
        out[b, :, h * 32 : (h + 1) * 32, :] = res.results[core]["out"].reshape(
            O, 32, W
        )
    return out


def _kernel_numpy(x, offset, weight):
    """Exact CPU fallback (same math as the device kernel, fp32)."""
    out = np.zeros((B, O, H, W), np.float32)
    Kh = Kw = 3
    ki = np.repeat(np.arange(Kh), Kw)
    kj = np.tile(np.arange(Kw), Kh)
    for b in range(B):
        xf = x[b].reshape(C, H * W)
        off = offset[b].reshape(K, 2, H, W)
        ho = np.arange(H)[None, :, None]
        wo = np.arange(W)[None, None, :]
        py = ho - 1 + ki[:, None, None] + off[:, 0]
        px = wo - 1 + kj[:, None, None] + off[:, 1]
        y0 = np.floor(py).astype(np.int64)
        x0 = np.floor(px).astype(np.int64)
        ly = (py - y0).astype(np.float32)
        lx = (px - x0).astype(np.float32)
        cols = np.zeros((C, K, H * W), np.float32)
        for dy in (0, 1):
            for dx in (0, 1):
                yy = y0 + dy
                xx = x0 + dx
                valid = (yy >= 0) & (yy < H) & (xx >= 0) & (xx < W)
                idx = np.clip(yy, 0, H - 1) * W + np.clip(xx, 0, W - 1)
                wgt = (ly if dy else 1 - ly) * (lx if dx else 1 - lx) * valid
                cols += xf[:, idx.reshape(K, -1)] * wgt.reshape(1, K, -1)
        out[b] = (
            weight.reshape(O, C, K).transpose(0, 2, 1).reshape(O, K * C)
            @ cols.transpose(1, 0, 2).reshape(K * C, H * W)
        ).reshape(O, H, W)
    return out


_KERNEL_FAILED = False


def kernel(x, offset, weight):
    global _KERNEL_FAILED
    x = np.ascontiguousarray(np.asarray(x, np.float32))
    offset = np.ascontiguousarray(np.asarray(offset, np.float32))
    weight = np.ascontiguousarray(np.asarray(weight, np.float32))
    if not _KERNEL_FAILED:
        try:
            return _kernel_device(x, offset, weight)
        except Exception as e:
            import sys

            print(f"device kernel failed ({type(e).__name__}: {e}); "
                  "falling back to CPU", file=sys.stderr)
            _KERNEL_FAILED = True
    return _kernel_numpy(x, offset, weight)
